# revision 22
# baseline (speedup 1.0000x reference)
"""Trainium2 Bass kernel for ACLIP top-k patch masking.

Reference computation (per batch):
    cls, patches = split(image_features)            # [1,D], [P,D]  P=576
    sim = normalize(patches) @ normalize(text)      # [P]
    idx = sort(top_k(sim, 288).indices)             # [288]
    out = concat([cls, patches[idx]])               # [289, D]

Distribution: pure data parallel, batch 256 -> 32 per core x 8 cores.

Per-core algorithm (B=32 batches, P=576 patches, D=1024, K=288):
  - Load patch rows [128, 5, 1024] per batch (chunk 4 half-filled).
  - prod = X * text_bcast (text norm is a positive per-batch constant and
    cannot change the top-k ordering, so text is used unnormalized).
  - s[p] = sum_d prod[p, d], n[p] = sum_d X[p, d]^2.
  - u[p] = sign(s) * s^2 / n, a monotone transform of the cosine sim
    (avoids sqrt, which only exists on the scalar engine).
  - rank[p] = #{q: u[q] > u[p]} exactly, comparing each u-column against
    a PSUM row of all 576 sims built by a diag(u)-matmul broadcast.
    DVE chunks: tensor_scalar is_gt + accum. ACT chunks: Sign(u_p - u_q)
    + accum gives 575 - 2*rank, so keep (rank < K) becomes signsum >= 0.
  - dest slot = within-chunk cumsum (triangular matmul) + chunk offsets,
    accumulated into one PSUM tile. Kept rows are written by an indirect
    scatter DMA to rows [b*289+1 ...]; dropped rows get dest=1e6 and
    tie-overflow slots exceed the DMA bounds check, so both are skipped.
    CLS rows go by a strided DMA.

The per-batch chain is long (load -> sims -> rank -> scatter), so the
emission order is software-pipelined: stage A (loads + streaming sims)
for batch b is emitted alongside stage B (rank + scatter) for batch
b-LAG, which keeps every engine's in-order queue from serializing
consecutive batches.
"""

import numpy as np

import concourse.bass as bass
import concourse.mybir as mybir
import concourse.tile as tile
from concourse import bacc
from concourse.bass import IndirectOffsetOnAxis
from concourse.masks import make_identity, make_upper_triangular

F32 = mybir.dt.float32
I32 = mybir.dt.int32

B_FULL = 256
N_CORES = 8
B_CORE = B_FULL // N_CORES
NUM_TOKENS = 577
P = 576          # patches per batch
D = 1024
K = 288          # kept patches
OUT_TOK = K + 1  # cls + kept
NCH = 5          # 128-row chunks per batch (4 full + 1 of 64)
LAST = P - 4 * 128  # rows in last chunk = 64
# Skip sentinel for dropped rows. Must be f32-exact, > any valid row index,
# and small enough that sentinel * D stays within int32 (the indirect DMA
# multiplies indices by the row stride).
BIG = 1.0e6

RANK_ACT = (0, 1, 2)  # rank chunks on ACT (prefix of 0..3; never chunk 4)
LAG = 4            # stage-B emission lag (batches)


def _stage_a(nc, pools, img, txt, b):
    """Loads + streaming sims: prod, s, n."""
    (xpool, prpool, bcpool, spool, jpool, trpool, dgpool,
     pprow, pprbc, ppcum) = pools
    st = {}
    x = xpool.tile([128, NCH, D], F32, tag="x")
    nc.sync.dma_start(
        out=x[:, 0:4, :],
        in_=img[b, 1:513, :].rearrange("(c p) d -> p c d", p=128),
    )
    nc.sync.dma_start(out=x[0:LAST, 4, :], in_=img[b, 513:577, :])
    st["x"] = x

    txtb = bcpool.tile([128, D], F32, tag="txtb")
    nc.sync.dma_start(out=txtb[0:1, :], in_=txt[b : b + 1, :])
    w = 1
    while w < 128:
        nc.sync.dma_start(out=txtb[w : 2 * w, :], in_=txtb[0:w, :])
        w *= 2

    S = spool.tile([128, NCH], F32, tag="S")
    N = spool.tile([128, NCH], F32, tag="N")
    nc.vector.memset(S[LAST:128, 4:5], 0.0)
    nc.vector.memset(N[LAST:128, 4:5], 1.0)
    st["S"], st["N"] = S, N

    prod = prpool.tile([128, 4, D], F32, tag="prod")
    nc.vector.tensor_tensor(
        out=prod[:, :, :], in0=x[:, 0:4, :],
        in1=txtb[:, None, :].to_broadcast([128, 4, D]),
        op=mybir.AluOpType.mult,
    )
    nc.vector.tensor_reduce(
        out=S[:, 0:4], in_=prod[:, :, :],
        axis=mybir.AxisListType.X, op=mybir.AluOpType.add,
    )
    prod4 = prpool.tile([128, D], F32, tag="prod4")
    nc.gpsimd.tensor_tensor(
        out=prod4[0:LAST, :], in0=x[0:LAST, 4, :],
        in1=txtb[0:LAST, :], op=mybir.AluOpType.mult,
    )
    ja = jpool.tile([128, D], F32, tag="ja")
    nc.scalar.activation(
        out=ja[0:LAST, :], in_=prod4[0:LAST, :],
        func=mybir.ActivationFunctionType.Copy,
        accum_out=S[0:LAST, 4:5],
    )
    for c in range(NCH):
        rows = 128 if c < 4 else LAST
        js = jpool.tile([128, D], F32, tag="ja")
        nc.scalar.activation(
            out=js[:rows, :], in_=x[:rows, c, :],
            func=mybir.ActivationFunctionType.Square,
            accum_out=N[:rows, c : c + 1],
        )
    return st


def _stage_b(nc, pools, consts, out_flat, b, st):
    """u, ranks, destinations, scatter."""
    (xpool, prpool, bcpool, spool, jpool, trpool, dgpool,
     pprow, pprbc, ppcum) = pools
    ident, ltri, ones_col, ones_row, ones_mat = consts
    x, S, N = st["x"], st["S"], st["N"]

    # ---- u = sign(s) * s^2 / n  (monotone in the cosine sim) ----
    SS = spool.tile([128, NCH], F32, tag="SS")
    nc.vector.tensor_tensor(out=SS[:], in0=S[:], in1=S[:],
                            op=mybir.AluOpType.mult)
    REC = spool.tile([128, NCH], F32, tag="REC")
    nc.vector.reciprocal(REC[:], N[:])
    UA = spool.tile([128, NCH], F32, tag="UA")
    nc.vector.tensor_tensor(out=UA[:], in0=SS[:], in1=REC[:],
                            op=mybir.AluOpType.mult)
    SGN = spool.tile([128, NCH], I32, tag="SGN")
    nc.vector.tensor_scalar(
        out=SGN[:], in0=S[:].bitcast(I32), scalar1=-0x80000000,
        scalar2=None, op0=mybir.AluOpType.bitwise_and,
    )
    U = spool.tile([128, NCH], F32, tag="U")
    nc.vector.tensor_tensor(
        out=U[:].bitcast(I32), in0=UA[:].bitcast(I32), in1=SGN[:],
        op=mybir.AluOpType.bitwise_or,
    )
    # garbage rows of the half chunk must never rank into top-K
    nc.vector.memset(U[LAST:128, 4:5], -1e30)

    # ---- all-sims row in PSUM via one matmul per chunk:
    # rbc[p, j] = sum_k ones[k, p] * diag(u_col)[k, j] = u[j]
    rbcps = pprbc.tile([128, P], F32, tag="rbcps")
    for c in range(NCH):
        w = 128 if c < 4 else LAST
        diagU = dgpool.tile([128, 128], F32, tag="diagU")
        nc.vector.tensor_scalar(
            out=diagU[:], in0=ident[:],
            scalar1=U[:, c : c + 1], scalar2=None,
            op0=mybir.AluOpType.mult,
        )
        nc.tensor.matmul(
            rbcps[:, c * 128 : c * 128 + w],
            lhsT=ones_mat[:],
            rhs=diagU[:, 0:w],
            start=True, stop=True,
        )

    # ---- exact ranks ----
    RANK = spool.tile([128, NCH], F32, tag="RANK")
    for c in range(NCH):
        rows = 128 if c < 4 else LAST
        if c in RANK_ACT:
            jr = jpool.tile([128, P], F32, tag="jract")
            nc.scalar.activation(
                out=jr[:rows, :], in_=rbcps[:rows, :],
                func=mybir.ActivationFunctionType.Sign,
                bias=U[:rows, c : c + 1], scale=-1.0,
                accum_out=RANK[:rows, c : c + 1],
            )
        else:
            jr = jpool.tile([128, P], F32, tag="jrdve")
            nc.vector.tensor_scalar(
                out=jr[:rows, :], in0=rbcps[:rows, :],
                scalar1=U[:rows, c : c + 1], scalar2=0.0,
                op0=mybir.AluOpType.is_gt,
                op1=mybir.AluOpType.add,
                accum_out=RANK[:rows, c : c + 1],
            )
    nc.vector.memset(RANK[LAST:128, 4:5], 1e9)

    # ---- keep mask ----
    mask = spool.tile([128, NCH], F32, tag="mask")
    na = len(RANK_ACT)
    if na:
        nc.vector.tensor_scalar(
            out=mask[:, 0:na], in0=RANK[:, 0:na],
            scalar1=0.0, scalar2=None, op0=mybir.AluOpType.is_ge,
        )
    nc.vector.tensor_scalar(
        out=mask[:, na:NCH], in0=RANK[:, na:NCH],
        scalar1=float(K), scalar2=None, op0=mybir.AluOpType.is_lt,
    )

    # exclusive per-chunk offsets from the chunk totals
    tpsum = pprow.tile([1, NCH], F32, tag="rpsum")
    nc.tensor.matmul(tpsum[:], lhsT=ones_col[:], rhs=mask[:],
                     start=True, stop=True)
    tot = spool.tile([1, NCH], F32, tag="tot")
    nc.vector.tensor_copy(tot[:], tpsum[:])
    oinc = spool.tile([1, NCH], F32, tag="oinc")
    nc.vector.tensor_tensor_scan(
        out=oinc[:], data0=tot[:], data1=tot[:], initial=0.0,
        op0=mybir.AluOpType.add, op1=mybir.AluOpType.bypass,
    )
    offx = spool.tile([1, NCH], F32, tag="offx")
    nc.vector.tensor_tensor(out=offx[:], in0=oinc[:], in1=tot[:],
                            op=mybir.AluOpType.subtract)

    # G = within-chunk cumsum + chunk offset, accumulated in PSUM
    cpsum = ppcum.tile([128, NCH], F32, tag="cpsum")
    nc.tensor.matmul(cpsum[:], lhsT=ltri[:], rhs=mask[:],
                     start=True, stop=False)
    nc.tensor.matmul(cpsum[:], lhsT=ones_row[:], rhs=offx[:1, :],
                     start=False, stop=True)

    # dest = mask ? G + b*289 : BIG (tie overflow handled by bounds check)
    W = spool.tile([128, NCH], F32, tag="W")
    nc.vector.tensor_scalar(
        out=W[:], in0=mask[:], scalar1=-BIG,
        scalar2=BIG + float(b * OUT_TOK),
        op0=mybir.AluOpType.mult, op1=mybir.AluOpType.add,
    )
    DF = spool.tile([128, NCH], F32, tag="DF")
    nc.vector.tensor_tensor(out=DF[:], in0=cpsum[:], in1=W[:],
                            op=mybir.AluOpType.add)
    desti = spool.tile([128, NCH], I32, tag="desti")
    nc.vector.tensor_copy(out=desti[:], in_=DF[:])

    # ---- scatter kept rows (one offset column per chunk) ----
    for c in range(NCH):
        rows = 128 if c < 4 else LAST
        nc.gpsimd.indirect_dma_start(
            out=out_flat[:, :],
            out_offset=IndirectOffsetOnAxis(
                ap=desti[0:rows, c : c + 1], axis=0
            ),
            in_=x[0:rows, c, :],
            in_offset=None,
            bounds_check=b * OUT_TOK + K,
            oob_is_err=False,
        )


def build(nc, b_core=B_CORE, img=None, txt=None, out=None):
    if img is None:
        img = nc.dram_tensor("image_features", [b_core, NUM_TOKENS, D], F32,
                             kind="ExternalInput").ap()
        txt = nc.dram_tensor("text_features", [b_core, D], F32,
                             kind="ExternalInput").ap()
        out = nc.dram_tensor("out", [b_core, OUT_TOK, D], F32,
                             kind="ExternalOutput").ap()

    out_flat = out.rearrange("b k d -> (b k) d")

    with tile.TileContext(nc) as tc:
        with (
            tc.tile_pool(name="consts", bufs=1) as cpool,
            tc.tile_pool(name="x", bufs=6) as xpool,
            tc.tile_pool(name="prod", bufs=2) as prpool,
            tc.tile_pool(name="bcast", bufs=3) as bcpool,
            tc.tile_pool(name="small", bufs=8) as spool,
            tc.tile_pool(name="junk", bufs=2) as jpool,
            tc.tile_pool(name="trow", bufs=3) as trpool,
            tc.tile_pool(name="diag", bufs=4) as dgpool,
            tc.tile_pool(name="ps_row", bufs=2, space="PSUM") as pprow,
            tc.tile_pool(name="ps_rbc", bufs=2, space="PSUM") as pprbc,
            tc.tile_pool(name="ps_cum", bufs=2, space="PSUM") as ppcum,
        ):
            ident = cpool.tile([128, 128], F32)
            make_identity(nc, ident[:])
            ltri = cpool.tile([128, 128], F32)
            make_upper_triangular(nc, ltri[:], val=1.0, diag=True)
            ones_col = cpool.tile([128, 1], F32)
            nc.vector.memset(ones_col[:], 1.0)
            ones_row = cpool.tile([1, 128], F32)
            nc.vector.memset(ones_row[:], 1.0)
            ones_mat = cpool.tile([128, 128], F32)
            nc.vector.memset(ones_mat[:], 1.0)

            # CLS passthrough for all batches (SBUF bounce).
            clsbuf = cpool.tile([b_core, D], F32)
            nc.sync.dma_start(out=clsbuf[:], in_=img[:, 0, :])
            nc.sync.dma_start(out=out[:, 0, :], in_=clsbuf[:])

            pools = (xpool, prpool, bcpool, spool, jpool, trpool, dgpool,
                     pprow, pprbc, ppcum)
            consts = (ident, ltri, ones_col, ones_row, ones_mat)
            states = {}
            for i in range(b_core + LAG):
                if i < b_core:
                    states[i] = _stage_a(nc, pools, img, txt, i)
                j = i - LAG
                if j >= 0:
                    _stage_b(nc, pools, consts, out_flat, j, states.pop(j))
    return nc


_CACHED = {}


def _get_nc():
    if "nc" not in _CACHED:
        nc = bacc.Bacc("TRN2", target_bir_lowering=False)
        build(nc)
        nc.compile()
        _CACHED["nc"] = nc
    return _CACHED["nc"]


LAST_RESULT = None


def kernel(image_features, text_features):
    global LAST_RESULT
    from concourse.bass_utils import run_bass_kernel_spmd

    img = np.ascontiguousarray(np.asarray(image_features, dtype=np.float32))
    txt = np.ascontiguousarray(np.asarray(text_features, dtype=np.float32))
    assert img.shape == (B_FULL, NUM_TOKENS, D)
    assert txt.shape == (B_FULL, D)

    nc = _get_nc()
    in_maps = [
        {
            "image_features": img[i * B_CORE : (i + 1) * B_CORE],
            "text_features": txt[i * B_CORE : (i + 1) * B_CORE],
        }
        for i in range(N_CORES)
    ]
    res = run_bass_kernel_spmd(nc, in_maps, core_ids=list(range(N_CORES)))
    LAST_RESULT = res
    return np.concatenate([res.results[i]["out"] for i in range(N_CORES)], axis=0)


# revision 23
# speedup vs baseline: 1.0084x; 1.0084x over previous
"""Trainium2 Bass kernel for ACLIP top-k patch masking.

Reference computation (per batch):
    cls, patches = split(image_features)            # [1,D], [P,D]  P=576
    sim = normalize(patches) @ normalize(text)      # [P]
    idx = sort(top_k(sim, 288).indices)             # [288]
    out = concat([cls, patches[idx]])               # [289, D]

Distribution: pure data parallel, batch 256 -> 32 per core x 8 cores.

Per-core algorithm (B=32 batches, P=576 patches, D=1024, K=288):
  - Load patch rows [128, 5, 1024] per batch (chunk 4 half-filled).
  - prod = X * text_bcast (text norm is a positive per-batch constant and
    cannot change the top-k ordering, so text is used unnormalized).
  - s[p] = sum_d prod[p, d], n[p] = sum_d X[p, d]^2.
  - u[p] = sign(s) * s^2 / n, a monotone transform of the cosine sim
    (avoids sqrt, which only exists on the scalar engine).
  - rank[p] = #{q: u[q] > u[p]} exactly, comparing each u-column against
    a PSUM row of all 576 sims built by a diag(u)-matmul broadcast.
    DVE chunks: tensor_scalar is_gt + accum. ACT chunks: Sign(u_p - u_q)
    + accum gives 575 - 2*rank, so keep (rank < K) becomes signsum >= 0.
  - dest slot = within-chunk cumsum (triangular matmul) + chunk offsets,
    accumulated into one PSUM tile. Kept rows are written by an indirect
    scatter DMA to rows [b*289+1 ...]; dropped rows get dest=1e6 and
    tie-overflow slots exceed the DMA bounds check, so both are skipped.
    CLS rows go by a strided DMA.

The per-batch chain is long (load -> sims -> rank -> scatter), so the
emission order is software-pipelined: stage A (loads + streaming sims)
for batch b is emitted alongside stage B (rank + scatter) for batch
b-LAG, which keeps every engine's in-order queue from serializing
consecutive batches.
"""

import numpy as np

import concourse.bass as bass
import concourse.mybir as mybir
import concourse.tile as tile
from concourse import bacc
from concourse.bass import IndirectOffsetOnAxis
from concourse.masks import make_identity, make_upper_triangular

F32 = mybir.dt.float32
I32 = mybir.dt.int32

B_FULL = 256
N_CORES = 8
B_CORE = B_FULL // N_CORES
NUM_TOKENS = 577
P = 576          # patches per batch
D = 1024
K = 288          # kept patches
OUT_TOK = K + 1  # cls + kept
NCH = 5          # 128-row chunks per batch (4 full + 1 of 64)
LAST = P - 4 * 128  # rows in last chunk = 64
# Skip sentinel for dropped rows. Must be f32-exact, > any valid row index,
# and small enough that sentinel * D stays within int32 (the indirect DMA
# multiplies indices by the row stride).
BIG = 1.0e6

RANK_ACT = (0, 1, 2)  # rank chunks on ACT (prefix of 0..3; never chunk 4)
LAG = 3            # stage-B emission lag (batches)


def _stage_a(nc, pools, img, txt, b):
    """Loads + streaming sims: prod, s, n."""
    (xpool, prpool, bcpool, spool, jpool, trpool, dgpool,
     pprow, pprbc, ppcum) = pools
    st = {}
    x = xpool.tile([128, NCH, D], F32, tag="x")
    nc.sync.dma_start(
        out=x[:, 0:4, :],
        in_=img[b, 1:513, :].rearrange("(c p) d -> p c d", p=128),
    )
    nc.sync.dma_start(out=x[0:LAST, 4, :], in_=img[b, 513:577, :])
    st["x"] = x

    txtb = bcpool.tile([128, D], F32, tag="txtb")
    nc.sync.dma_start(out=txtb[0:1, :], in_=txt[b : b + 1, :])
    w = 1
    while w < 128:
        nc.sync.dma_start(out=txtb[w : 2 * w, :], in_=txtb[0:w, :])
        w *= 2

    S = spool.tile([128, NCH], F32, tag="S")
    N = spool.tile([128, NCH], F32, tag="N")
    nc.vector.memset(S[LAST:128, 4:5], 0.0)
    nc.vector.memset(N[LAST:128, 4:5], 1.0)
    st["S"], st["N"] = S, N

    prod = prpool.tile([128, 4, D], F32, tag="prod")
    nc.vector.tensor_tensor(
        out=prod[:, :, :], in0=x[:, 0:4, :],
        in1=txtb[:, None, :].to_broadcast([128, 4, D]),
        op=mybir.AluOpType.mult,
    )
    nc.vector.tensor_reduce(
        out=S[:, 0:4], in_=prod[:, :, :],
        axis=mybir.AxisListType.X, op=mybir.AluOpType.add,
    )
    prod4 = prpool.tile([128, D], F32, tag="prod4")
    nc.gpsimd.tensor_tensor(
        out=prod4[0:LAST, :], in0=x[0:LAST, 4, :],
        in1=txtb[0:LAST, :], op=mybir.AluOpType.mult,
    )
    ja = jpool.tile([128, D], F32, tag="ja")
    nc.scalar.activation(
        out=ja[0:LAST, :], in_=prod4[0:LAST, :],
        func=mybir.ActivationFunctionType.Copy,
        accum_out=S[0:LAST, 4:5],
    )
    for c in range(NCH):
        rows = 128 if c < 4 else LAST
        js = jpool.tile([128, D], F32, tag="ja")
        nc.scalar.activation(
            out=js[:rows, :], in_=x[:rows, c, :],
            func=mybir.ActivationFunctionType.Square,
            accum_out=N[:rows, c : c + 1],
        )
    return st


def _stage_b(nc, pools, consts, out_flat, b, st):
    """u, ranks, destinations, scatter."""
    (xpool, prpool, bcpool, spool, jpool, trpool, dgpool,
     pprow, pprbc, ppcum) = pools
    ident, ltri, ones_col, ones_row, ones_mat = consts
    x, S, N = st["x"], st["S"], st["N"]

    # ---- u = sign(s) * s^2 / n  (monotone in the cosine sim) ----
    SS = spool.tile([128, NCH], F32, tag="SS")
    nc.vector.tensor_tensor(out=SS[:], in0=S[:], in1=S[:],
                            op=mybir.AluOpType.mult)
    REC = spool.tile([128, NCH], F32, tag="REC")
    nc.vector.reciprocal(REC[:], N[:])
    UA = spool.tile([128, NCH], F32, tag="UA")
    nc.vector.tensor_tensor(out=UA[:], in0=SS[:], in1=REC[:],
                            op=mybir.AluOpType.mult)
    SGN = spool.tile([128, NCH], I32, tag="SGN")
    nc.vector.tensor_scalar(
        out=SGN[:], in0=S[:].bitcast(I32), scalar1=-0x80000000,
        scalar2=None, op0=mybir.AluOpType.bitwise_and,
    )
    U = spool.tile([128, NCH], F32, tag="U")
    nc.vector.tensor_tensor(
        out=U[:].bitcast(I32), in0=UA[:].bitcast(I32), in1=SGN[:],
        op=mybir.AluOpType.bitwise_or,
    )
    # garbage rows of the half chunk must never rank into top-K
    nc.vector.memset(U[LAST:128, 4:5], -1e30)

    # ---- all-sims row in PSUM via one matmul per chunk:
    # rbc[p, j] = sum_k ones[k, p] * diag(u_col)[k, j] = u[j]
    rbcps = pprbc.tile([128, P], F32, tag="rbcps")
    for c in range(NCH):
        w = 128 if c < 4 else LAST
        diagU = dgpool.tile([128, 128], F32, tag="diagU")
        nc.vector.tensor_scalar(
            out=diagU[:], in0=ident[:],
            scalar1=U[:, c : c + 1], scalar2=None,
            op0=mybir.AluOpType.mult,
        )
        nc.tensor.matmul(
            rbcps[:, c * 128 : c * 128 + w],
            lhsT=ones_mat[:],
            rhs=diagU[:, 0:w],
            start=True, stop=True,
        )

    # ---- exact ranks ----
    RANK = spool.tile([128, NCH], F32, tag="RANK")
    for c in range(NCH):
        rows = 128 if c < 4 else LAST
        if c in RANK_ACT:
            jr = jpool.tile([128, P], F32, tag="jract")
            nc.scalar.activation(
                out=jr[:rows, :], in_=rbcps[:rows, :],
                func=mybir.ActivationFunctionType.Sign,
                bias=U[:rows, c : c + 1], scale=-1.0,
                accum_out=RANK[:rows, c : c + 1],
            )
        else:
            jr = jpool.tile([128, P], F32, tag="jrdve")
            nc.vector.tensor_scalar(
                out=jr[:rows, :], in0=rbcps[:rows, :],
                scalar1=U[:rows, c : c + 1], scalar2=0.0,
                op0=mybir.AluOpType.is_gt,
                op1=mybir.AluOpType.add,
                accum_out=RANK[:rows, c : c + 1],
            )
    nc.vector.memset(RANK[LAST:128, 4:5], 1e9)

    # ---- keep mask ----
    mask = spool.tile([128, NCH], F32, tag="mask")
    na = len(RANK_ACT)
    if na:
        nc.vector.tensor_scalar(
            out=mask[:, 0:na], in0=RANK[:, 0:na],
            scalar1=0.0, scalar2=None, op0=mybir.AluOpType.is_ge,
        )
    nc.vector.tensor_scalar(
        out=mask[:, na:NCH], in0=RANK[:, na:NCH],
        scalar1=float(K), scalar2=None, op0=mybir.AluOpType.is_lt,
    )

    # exclusive per-chunk offsets from the chunk totals
    tpsum = pprow.tile([1, NCH], F32, tag="rpsum")
    nc.tensor.matmul(tpsum[:], lhsT=ones_col[:], rhs=mask[:],
                     start=True, stop=True)
    tot = spool.tile([1, NCH], F32, tag="tot")
    nc.vector.tensor_copy(tot[:], tpsum[:])
    oinc = spool.tile([1, NCH], F32, tag="oinc")
    nc.vector.tensor_tensor_scan(
        out=oinc[:], data0=tot[:], data1=tot[:], initial=0.0,
        op0=mybir.AluOpType.add, op1=mybir.AluOpType.bypass,
    )
    offx = spool.tile([1, NCH], F32, tag="offx")
    nc.vector.tensor_tensor(out=offx[:], in0=oinc[:], in1=tot[:],
                            op=mybir.AluOpType.subtract)

    # G = within-chunk cumsum + chunk offset, accumulated in PSUM
    cpsum = ppcum.tile([128, NCH], F32, tag="cpsum")
    nc.tensor.matmul(cpsum[:], lhsT=ltri[:], rhs=mask[:],
                     start=True, stop=False)
    nc.tensor.matmul(cpsum[:], lhsT=ones_row[:], rhs=offx[:1, :],
                     start=False, stop=True)

    # dest = mask ? G + b*289 : BIG (tie overflow handled by bounds check)
    W = spool.tile([128, NCH], F32, tag="W")
    nc.vector.tensor_scalar(
        out=W[:], in0=mask[:], scalar1=-BIG,
        scalar2=BIG + float(b * OUT_TOK),
        op0=mybir.AluOpType.mult, op1=mybir.AluOpType.add,
    )
    DF = spool.tile([128, NCH], F32, tag="DF")
    nc.vector.tensor_tensor(out=DF[:], in0=cpsum[:], in1=W[:],
                            op=mybir.AluOpType.add)
    desti = spool.tile([128, NCH], I32, tag="desti")
    nc.vector.tensor_copy(out=desti[:], in_=DF[:])

    # ---- scatter kept rows (one offset column per chunk) ----
    for c in range(NCH):
        rows = 128 if c < 4 else LAST
        nc.gpsimd.indirect_dma_start(
            out=out_flat[:, :],
            out_offset=IndirectOffsetOnAxis(
                ap=desti[0:rows, c : c + 1], axis=0
            ),
            in_=x[0:rows, c, :],
            in_offset=None,
            bounds_check=b * OUT_TOK + K,
            oob_is_err=False,
        )


def build(nc, b_core=B_CORE, img=None, txt=None, out=None):
    if img is None:
        img = nc.dram_tensor("image_features", [b_core, NUM_TOKENS, D], F32,
                             kind="ExternalInput").ap()
        txt = nc.dram_tensor("text_features", [b_core, D], F32,
                             kind="ExternalInput").ap()
        out = nc.dram_tensor("out", [b_core, OUT_TOK, D], F32,
                             kind="ExternalOutput").ap()

    out_flat = out.rearrange("b k d -> (b k) d")

    with tile.TileContext(nc) as tc:
        with (
            tc.tile_pool(name="consts", bufs=1) as cpool,
            tc.tile_pool(name="x", bufs=5) as xpool,
            tc.tile_pool(name="prod", bufs=2) as prpool,
            tc.tile_pool(name="bcast", bufs=3) as bcpool,
            tc.tile_pool(name="small", bufs=8) as spool,
            tc.tile_pool(name="junk", bufs=3) as jpool,
            tc.tile_pool(name="trow", bufs=3) as trpool,
            tc.tile_pool(name="diag", bufs=4) as dgpool,
            tc.tile_pool(name="ps_row", bufs=2, space="PSUM") as pprow,
            tc.tile_pool(name="ps_rbc", bufs=2, space="PSUM") as pprbc,
            tc.tile_pool(name="ps_cum", bufs=2, space="PSUM") as ppcum,
        ):
            ident = cpool.tile([128, 128], F32)
            make_identity(nc, ident[:])
            ltri = cpool.tile([128, 128], F32)
            make_upper_triangular(nc, ltri[:], val=1.0, diag=True)
            ones_col = cpool.tile([128, 1], F32)
            nc.vector.memset(ones_col[:], 1.0)
            ones_row = cpool.tile([1, 128], F32)
            nc.vector.memset(ones_row[:], 1.0)
            ones_mat = cpool.tile([128, 128], F32)
            nc.vector.memset(ones_mat[:], 1.0)

            # CLS passthrough for all batches (SBUF bounce).
            clsbuf = cpool.tile([b_core, D], F32)
            nc.sync.dma_start(out=clsbuf[:], in_=img[:, 0, :])
            nc.sync.dma_start(out=out[:, 0, :], in_=clsbuf[:])

            pools = (xpool, prpool, bcpool, spool, jpool, trpool, dgpool,
                     pprow, pprbc, ppcum)
            consts = (ident, ltri, ones_col, ones_row, ones_mat)
            states = {}
            for i in range(b_core + LAG):
                if i < b_core:
                    states[i] = _stage_a(nc, pools, img, txt, i)
                j = i - LAG
                if j >= 0:
                    _stage_b(nc, pools, consts, out_flat, j, states.pop(j))
    return nc


_CACHED = {}


def _get_nc():
    if "nc" not in _CACHED:
        nc = bacc.Bacc("TRN2", target_bir_lowering=False)
        build(nc)
        nc.compile()
        _CACHED["nc"] = nc
    return _CACHED["nc"]


LAST_RESULT = None


def kernel(image_features, text_features):
    global LAST_RESULT
    from concourse.bass_utils import run_bass_kernel_spmd

    img = np.ascontiguousarray(np.asarray(image_features, dtype=np.float32))
    txt = np.ascontiguousarray(np.asarray(text_features, dtype=np.float32))
    assert img.shape == (B_FULL, NUM_TOKENS, D)
    assert txt.shape == (B_FULL, D)

    nc = _get_nc()
    in_maps = [
        {
            "image_features": img[i * B_CORE : (i + 1) * B_CORE],
            "text_features": txt[i * B_CORE : (i + 1) * B_CORE],
        }
        for i in range(N_CORES)
    ]
    res = run_bass_kernel_spmd(nc, in_maps, core_ids=list(range(N_CORES)))
    LAST_RESULT = res
    return np.concatenate([res.results[i]["out"] for i in range(N_CORES)], axis=0)


# revision 24
# speedup vs baseline: 1.2367x; 1.2264x over previous
"""Trainium2 Bass kernel for ACLIP top-k patch masking.

Reference computation (per batch):
    cls, patches = split(image_features)            # [1,D], [P,D]  P=576
    sim = normalize(patches) @ normalize(text)      # [P]
    idx = sort(top_k(sim, 288).indices)             # [288]
    out = concat([cls, patches[idx]])               # [289, D]

Distribution: pure data parallel, batch 256 -> 32 per core x 8 cores.

Per-core algorithm (B=32 batches, P=576 patches, D=1024, K=288):
  - Load patch rows [128, 5, 1024] per batch (chunk 4 half-filled).
  - prod = X * text_bcast (text norm is a positive per-batch constant and
    cannot change the top-k ordering, so text is used unnormalized).
  - s[p] = sum_d prod[p, d], n[p] = sum_d X[p, d]^2.
  - u[p] = sign(s) * s^2 / n, a monotone transform of the cosine sim
    (avoids sqrt, which only exists on the scalar engine).
  - rank[p] = #{q: u[q] > u[p]} exactly, comparing each u-column against
    a PSUM row of all 576 sims built by a diag(u)-matmul broadcast.
    DVE chunks: tensor_scalar is_gt + accum. ACT chunks: Sign(u_p - u_q)
    + accum gives 575 - 2*rank, so keep (rank < K) becomes signsum >= 0.
  - dest slot = within-chunk cumsum (triangular matmul) + chunk offsets,
    accumulated into one PSUM tile. Kept rows are written by an indirect
    scatter DMA to rows [b*289+1 ...]; dropped rows get dest=1e6 and
    tie-overflow slots exceed the DMA bounds check, so both are skipped.
    CLS rows go by a strided DMA.

The per-batch chain is long (load -> sims -> rank -> scatter), so the
emission order is software-pipelined: stage A (loads + streaming sims)
for batch b is emitted alongside stage B (rank + scatter) for batch
b-LAG, which keeps every engine's in-order queue from serializing
consecutive batches.
"""

import numpy as np

import concourse.bass as bass
import concourse.mybir as mybir
import concourse.tile as tile
from concourse import bacc
from concourse.bass import IndirectOffsetOnAxis
from concourse.masks import make_identity, make_upper_triangular

F32 = mybir.dt.float32
I32 = mybir.dt.int32

B_FULL = 256
N_CORES = 8
B_CORE = B_FULL // N_CORES
NUM_TOKENS = 577
P = 576          # patches per batch
D = 1024
K = 288          # kept patches
OUT_TOK = K + 1  # cls + kept
NCH = 5          # 128-row chunks per batch (4 full + 1 of 64)
LAST = P - 4 * 128  # rows in last chunk = 64
# Skip sentinel for dropped rows. Must be f32-exact, > any valid row index,
# and small enough that sentinel * D stays within int32 (the indirect DMA
# multiplies indices by the row stride).
BIG = 1.0e6

RANK_ACT = (0, 1, 2)  # rank chunks on ACT (prefix of 0..3; never chunk 4)
LAG = 3            # stage-B emission lag (batches)


def _stage_a(nc, pools, img, txt, b):
    """Loads + streaming sims: prod, s, n."""
    (xpool, prpool, bcpool, spool, jpool, trpool, dgpool,
     pprow, pprbc, ppcum) = pools
    st = {}
    x = xpool.tile([128, NCH, D], F32, tag="x")
    nc.sync.dma_start(
        out=x[:, 0:4, :],
        in_=img[b, 1:513, :].rearrange("(c p) d -> p c d", p=128),
    )
    nc.sync.dma_start(out=x[0:LAST, 4, :], in_=img[b, 513:577, :])
    st["x"] = x

    txtb = bcpool.tile([128, D], F32, tag="txtb")
    nc.sync.dma_start(out=txtb[0:1, :], in_=txt[b : b + 1, :])
    w = 1
    while w < 128:
        nc.sync.dma_start(out=txtb[w : 2 * w, :], in_=txtb[0:w, :])
        w *= 2

    S = spool.tile([128, NCH], F32, tag="S")
    N = spool.tile([128, NCH], F32, tag="N")
    nc.vector.memset(S[LAST:128, 4:5], 0.0)
    nc.vector.memset(N[LAST:128, 4:5], 1.0)
    st["S"], st["N"] = S, N

    prod = prpool.tile([128, 4, D], F32, tag="prod")
    nc.vector.tensor_tensor(
        out=prod[:, :, :], in0=x[:, 0:4, :],
        in1=txtb[:, None, :].to_broadcast([128, 4, D]),
        op=mybir.AluOpType.mult,
    )
    nc.vector.tensor_reduce(
        out=S[:, 0:4], in_=prod[:, :, :],
        axis=mybir.AxisListType.X, op=mybir.AluOpType.add,
    )
    prod4 = prpool.tile([128, D], F32, tag="prod4")
    nc.gpsimd.tensor_tensor(
        out=prod4[0:LAST, :], in0=x[0:LAST, 4, :],
        in1=txtb[0:LAST, :], op=mybir.AluOpType.mult,
    )
    ja = jpool.tile([128, D], F32, tag="ja")
    nc.scalar.activation(
        out=ja[0:LAST, :], in_=prod4[0:LAST, :],
        func=mybir.ActivationFunctionType.Copy,
        accum_out=S[0:LAST, 4:5],
    )
    for c in range(NCH):
        rows = 128 if c < 4 else LAST
        js = jpool.tile([128, D], F32, tag="ja")
        nc.scalar.activation(
            out=js[:rows, :], in_=x[:rows, c, :],
            func=mybir.ActivationFunctionType.Square,
            accum_out=N[:rows, c : c + 1],
        )
    return st


def _strip_out_waw(inst_h, prior_names):
    """Remove sync deps on earlier out-writers. All writers of `out` touch
    provably disjoint rows (CLS row 0; batch b's scatters only rows
    [b*289+1, b*289+288], slots unique via cumsum), so the completion-order
    WAW edges Tile inserts between them only serialize the DMA queue."""
    inst = getattr(inst_h, "ins", inst_h)
    for dep in list(inst.sync_dependency_names()):
        if dep in prior_names:
            try:
                inst.try_remove_dependency(dep)
            except Exception:
                inst.remove_dependency(dep)
    prior_names.add(inst.name)
    return inst


def _stage_b(nc, pools, consts, out_flat, b, st, out_writers):
    """u, ranks, destinations, scatter."""
    (xpool, prpool, bcpool, spool, jpool, trpool, dgpool,
     pprow, pprbc, ppcum) = pools
    ident, ltri, ones_col, ones_row, ones_mat = consts
    x, S, N = st["x"], st["S"], st["N"]

    # ---- u = sign(s) * s^2 / n  (monotone in the cosine sim) ----
    SS = spool.tile([128, NCH], F32, tag="SS")
    nc.vector.tensor_tensor(out=SS[:], in0=S[:], in1=S[:],
                            op=mybir.AluOpType.mult)
    REC = spool.tile([128, NCH], F32, tag="REC")
    nc.vector.reciprocal(REC[:], N[:])
    UA = spool.tile([128, NCH], F32, tag="UA")
    nc.vector.tensor_tensor(out=UA[:], in0=SS[:], in1=REC[:],
                            op=mybir.AluOpType.mult)
    SGN = spool.tile([128, NCH], I32, tag="SGN")
    nc.vector.tensor_scalar(
        out=SGN[:], in0=S[:].bitcast(I32), scalar1=-0x80000000,
        scalar2=None, op0=mybir.AluOpType.bitwise_and,
    )
    U = spool.tile([128, NCH], F32, tag="U")
    nc.vector.tensor_tensor(
        out=U[:].bitcast(I32), in0=UA[:].bitcast(I32), in1=SGN[:],
        op=mybir.AluOpType.bitwise_or,
    )
    # garbage rows of the half chunk must never rank into top-K
    nc.vector.memset(U[LAST:128, 4:5], -1e30)

    # ---- all-sims row in PSUM via one matmul per chunk:
    # rbc[p, j] = sum_k ones[k, p] * diag(u_col)[k, j] = u[j]
    rbcps = pprbc.tile([128, P], F32, tag="rbcps")
    for c in range(NCH):
        w = 128 if c < 4 else LAST
        diagU = dgpool.tile([128, 128], F32, tag="diagU")
        nc.vector.tensor_scalar(
            out=diagU[:], in0=ident[:],
            scalar1=U[:, c : c + 1], scalar2=None,
            op0=mybir.AluOpType.mult,
        )
        nc.tensor.matmul(
            rbcps[:, c * 128 : c * 128 + w],
            lhsT=ones_mat[:],
            rhs=diagU[:, 0:w],
            start=True, stop=True,
        )

    # ---- exact ranks ----
    RANK = spool.tile([128, NCH], F32, tag="RANK")
    for c in range(NCH):
        rows = 128 if c < 4 else LAST
        if c in RANK_ACT:
            jr = jpool.tile([128, P], F32, tag="jract")
            nc.scalar.activation(
                out=jr[:rows, :], in_=rbcps[:rows, :],
                func=mybir.ActivationFunctionType.Sign,
                bias=U[:rows, c : c + 1], scale=-1.0,
                accum_out=RANK[:rows, c : c + 1],
            )
        else:
            jr = jpool.tile([128, P], F32, tag="jrdve")
            nc.vector.tensor_scalar(
                out=jr[:rows, :], in0=rbcps[:rows, :],
                scalar1=U[:rows, c : c + 1], scalar2=0.0,
                op0=mybir.AluOpType.is_gt,
                op1=mybir.AluOpType.add,
                accum_out=RANK[:rows, c : c + 1],
            )
    nc.vector.memset(RANK[LAST:128, 4:5], 1e9)

    # ---- keep mask ----
    mask = spool.tile([128, NCH], F32, tag="mask")
    na = len(RANK_ACT)
    if na:
        nc.vector.tensor_scalar(
            out=mask[:, 0:na], in0=RANK[:, 0:na],
            scalar1=0.0, scalar2=None, op0=mybir.AluOpType.is_ge,
        )
    nc.vector.tensor_scalar(
        out=mask[:, na:NCH], in0=RANK[:, na:NCH],
        scalar1=float(K), scalar2=None, op0=mybir.AluOpType.is_lt,
    )

    # exclusive per-chunk offsets from the chunk totals
    tpsum = pprow.tile([1, NCH], F32, tag="rpsum")
    nc.tensor.matmul(tpsum[:], lhsT=ones_col[:], rhs=mask[:],
                     start=True, stop=True)
    tot = spool.tile([1, NCH], F32, tag="tot")
    nc.vector.tensor_copy(tot[:], tpsum[:])
    oinc = spool.tile([1, NCH], F32, tag="oinc")
    nc.vector.tensor_tensor_scan(
        out=oinc[:], data0=tot[:], data1=tot[:], initial=0.0,
        op0=mybir.AluOpType.add, op1=mybir.AluOpType.bypass,
    )
    offx = spool.tile([1, NCH], F32, tag="offx")
    nc.vector.tensor_tensor(out=offx[:], in0=oinc[:], in1=tot[:],
                            op=mybir.AluOpType.subtract)

    # G = within-chunk cumsum + chunk offset, accumulated in PSUM
    cpsum = ppcum.tile([128, NCH], F32, tag="cpsum")
    nc.tensor.matmul(cpsum[:], lhsT=ltri[:], rhs=mask[:],
                     start=True, stop=False)
    nc.tensor.matmul(cpsum[:], lhsT=ones_row[:], rhs=offx[:1, :],
                     start=False, stop=True)

    # dest = mask ? G + b*289 : BIG (tie overflow handled by bounds check)
    W = spool.tile([128, NCH], F32, tag="W")
    nc.vector.tensor_scalar(
        out=W[:], in0=mask[:], scalar1=-BIG,
        scalar2=BIG + float(b * OUT_TOK),
        op0=mybir.AluOpType.mult, op1=mybir.AluOpType.add,
    )
    DF = spool.tile([128, NCH], F32, tag="DF")
    nc.vector.tensor_tensor(out=DF[:], in0=cpsum[:], in1=W[:],
                            op=mybir.AluOpType.add)
    desti = spool.tile([128, NCH], I32, tag="desti")
    nc.vector.tensor_copy(out=desti[:], in_=DF[:])

    # ---- scatter kept rows (one offset column per chunk) ----
    for c in range(NCH):
        rows = 128 if c < 4 else LAST
        h = nc.gpsimd.indirect_dma_start(
            out=out_flat[:, :],
            out_offset=IndirectOffsetOnAxis(
                ap=desti[0:rows, c : c + 1], axis=0
            ),
            in_=x[0:rows, c, :],
            in_offset=None,
            bounds_check=b * OUT_TOK + K,
            oob_is_err=False,
        )
        _strip_out_waw(h, out_writers)


def build(nc, b_core=B_CORE, img=None, txt=None, out=None):
    if img is None:
        img = nc.dram_tensor("image_features", [b_core, NUM_TOKENS, D], F32,
                             kind="ExternalInput").ap()
        txt = nc.dram_tensor("text_features", [b_core, D], F32,
                             kind="ExternalInput").ap()
        out = nc.dram_tensor("out", [b_core, OUT_TOK, D], F32,
                             kind="ExternalOutput").ap()

    out_flat = out.rearrange("b k d -> (b k) d")

    with tile.TileContext(nc) as tc:
        with (
            tc.tile_pool(name="consts", bufs=1) as cpool,
            tc.tile_pool(name="x", bufs=5) as xpool,
            tc.tile_pool(name="prod", bufs=2) as prpool,
            tc.tile_pool(name="bcast", bufs=3) as bcpool,
            tc.tile_pool(name="small", bufs=8) as spool,
            tc.tile_pool(name="junk", bufs=3) as jpool,
            tc.tile_pool(name="trow", bufs=3) as trpool,
            tc.tile_pool(name="diag", bufs=4) as dgpool,
            tc.tile_pool(name="ps_row", bufs=2, space="PSUM") as pprow,
            tc.tile_pool(name="ps_rbc", bufs=2, space="PSUM") as pprbc,
            tc.tile_pool(name="ps_cum", bufs=2, space="PSUM") as ppcum,
        ):
            ident = cpool.tile([128, 128], F32)
            make_identity(nc, ident[:])
            ltri = cpool.tile([128, 128], F32)
            make_upper_triangular(nc, ltri[:], val=1.0, diag=True)
            ones_col = cpool.tile([128, 1], F32)
            nc.vector.memset(ones_col[:], 1.0)
            ones_row = cpool.tile([1, 128], F32)
            nc.vector.memset(ones_row[:], 1.0)
            ones_mat = cpool.tile([128, 128], F32)
            nc.vector.memset(ones_mat[:], 1.0)

            # CLS passthrough for all batches (SBUF bounce).
            out_writers = set()
            clsbuf = cpool.tile([b_core, D], F32)
            nc.sync.dma_start(out=clsbuf[:], in_=img[:, 0, :])
            hcls = nc.sync.dma_start(out=out[:, 0, :], in_=clsbuf[:])
            out_writers.add(getattr(hcls, "ins", hcls).name)

            pools = (xpool, prpool, bcpool, spool, jpool, trpool, dgpool,
                     pprow, pprbc, ppcum)
            consts = (ident, ltri, ones_col, ones_row, ones_mat)
            states = {}
            for i in range(b_core + LAG):
                if i < b_core:
                    states[i] = _stage_a(nc, pools, img, txt, i)
                j = i - LAG
                if j >= 0:
                    _stage_b(nc, pools, consts, out_flat, j, states.pop(j),
                             out_writers)
    return nc


_CACHED = {}


def _get_nc():
    if "nc" not in _CACHED:
        nc = bacc.Bacc("TRN2", target_bir_lowering=False)
        build(nc)
        nc.compile()
        _CACHED["nc"] = nc
    return _CACHED["nc"]


LAST_RESULT = None


def kernel(image_features, text_features):
    global LAST_RESULT
    from concourse.bass_utils import run_bass_kernel_spmd

    img = np.ascontiguousarray(np.asarray(image_features, dtype=np.float32))
    txt = np.ascontiguousarray(np.asarray(text_features, dtype=np.float32))
    assert img.shape == (B_FULL, NUM_TOKENS, D)
    assert txt.shape == (B_FULL, D)

    nc = _get_nc()
    in_maps = [
        {
            "image_features": img[i * B_CORE : (i + 1) * B_CORE],
            "text_features": txt[i * B_CORE : (i + 1) * B_CORE],
        }
        for i in range(N_CORES)
    ]
    res = run_bass_kernel_spmd(nc, in_maps, core_ids=list(range(N_CORES)))
    LAST_RESULT = res
    return np.concatenate([res.results[i]["out"] for i in range(N_CORES)], axis=0)
